# revision 16
# baseline (speedup 1.0000x reference)
"""Causal self-attention (B=2, T=2048, H=2048, NH=16) on 8 TRN2 NeuronCores.

Sharding: head-parallel. Core c owns heads 2c, 2c+1 (256 of 2048 qkv columns)
for both batch elements, computes its heads' attention output yT, AllGathers
yT across cores, then computes a 256-column slice of the output projection.

Layout choices (all matmuls transpose-free):
  - x is passed pre-transposed (xT: [H, B*T]) so projections contract
    feature-on-partition.
  - q/k are produced transposed ([hs, tok]) and RoPE is applied in
    rotate-half form: Wq/Wk columns are de-interleaved per head on the host.
  - scores are computed transposed (s^T[kt, q]) so the PV matmul and the
    output projection need no on-device transposes.
  - softmax denominator: DVE partition-block tree + ones-matmul + gpsimd
    partition broadcast.  Mask-after-softmax (no renorm) per the reference:
    full-row softmax, then tril mask; above-diagonal PV blocks are skipped.
"""

import sys

sys.path.insert(0, "/opt/trn_rl_repo")

import numpy as np
from contextlib import ExitStack

import concourse.mybir as mybir
import concourse.tile as tile
from concourse import bacc
from concourse.bass_utils import run_bass_kernel_spmd

B, T, H, NH, HS = 2, 2048, 2048, 16, 128
NCORES = 8
HPC = NH // NCORES        # heads per core: 2
CPC = HPC * HS            # qkv columns per core: 256
BT = B * T                # 4096
NKT = H // 128            # contraction k-tiles: 16
TKT = T // 128            # key tiles per batch: 16
NCH = BT // 512           # token 512-chunks: 8
QCH = T // 512            # q 512-chunks per batch: 4
F32 = mybir.dt.float32
F32R = mybir.dt.float32r
EXP = mybir.ActivationFunctionType.Exp
IDENT = mybir.ActivationFunctionType.Identity
SM_SCALE = float(1.0 / np.sqrt(HS))

_NC_CACHE = None
DEBUG = False


def _emit(nc):
    xT = nc.dram_tensor("xT", [H, BT], F32, kind="ExternalInput")
    wq = nc.dram_tensor("wq", [H, CPC], F32, kind="ExternalInput")
    wk = nc.dram_tensor("wk", [H, CPC], F32, kind="ExternalInput")
    wv = nc.dram_tensor("wv", [H, CPC], F32, kind="ExternalInput")
    bq2 = nc.dram_tensor("bq2", [128, HPC], F32, kind="ExternalInput")
    bk2 = nc.dram_tensor("bk2", [128, HPC], F32, kind="ExternalInput")
    bvb = nc.dram_tensor("bvb", [128, CPC], F32, kind="ExternalInput")
    wo = nc.dram_tensor("wo", [H, CPC], F32, kind="ExternalInput")
    bo2 = nc.dram_tensor("bo2", [128, HPC], F32, kind="ExternalInput")
    cosb = nc.dram_tensor("cosb", [128, BT], F32, kind="ExternalInput")
    sinb = nc.dram_tensor("sinb", [128, BT], F32, kind="ExternalInput")
    maskd = nc.dram_tensor("maskd", [128, 2048], F32, kind="ExternalInput")
    onesc = nc.dram_tensor("onesc", [128, 1], F32, kind="ExternalInput")
    oT = nc.dram_tensor("oT", [CPC, BT], F32, kind="ExternalOutput")

    yT_local = nc.dram_tensor("yT_local", [CPC, BT], F32)
    yT_all = nc.dram_tensor("yT_all", [H, BT], F32, addr_space="Shared")
    if DEBUG:
        dbg_q = nc.dram_tensor("dbg_q", [CPC, BT], F32, kind="ExternalOutput")
        dbg_k = nc.dram_tensor("dbg_k", [CPC, BT], F32, kind="ExternalOutput")
        dbg_v = nc.dram_tensor("dbg_v", [128, (BT // 128) * CPC], F32, kind="ExternalOutput")
        dbg_y = nc.dram_tensor("dbg_y", [CPC, BT], F32, kind="ExternalOutput")

    with tile.TileContext(nc) as tc, ExitStack() as ctx:
        persist = ctx.enter_context(tc.tile_pool(name="persist", bufs=1))

        # persistent SBUF: qT/kT per head, v, small bias tables
        qT = [
            persist.tile([128, BT], F32R, tag=f"qT{h}", name=f"qT{h}")
            for h in range(HPC)
        ]
        kT = [
            persist.tile([128, BT], F32R, tag=f"kT{h}", name=f"kT{h}")
            for h in range(HPC)
        ]
        v_sb = persist.tile([128, BT // 128, CPC], F32R, tag="v")
        bq_sb = persist.tile([128, HPC], F32, tag="bq")
        bk_sb = persist.tile([128, HPC], F32, tag="bk")
        bo_sb = persist.tile([128, HPC], F32, tag="bo")
        bvb_sb = persist.tile([128, CPC], F32, tag="bvb")
        ones_sb = persist.tile([128, 1], F32R, tag="ones")
        nc.sync.dma_start(bq_sb[:], bq2[:])
        nc.sync.dma_start(bk_sb[:], bk2[:])
        nc.sync.dma_start(bo_sb[:], bo2[:])
        nc.sync.dma_start(bvb_sb[:], bvb[:])
        nc.sync.dma_start(ones_sb[:], onesc[:].bitcast(F32R))

        # ---------------- Phase 1: QKV projections + RoPE ----------------
        with ExitStack() as c1:
            p1w = c1.enter_context(tc.tile_pool(name="p1w", bufs=1))
            p1s = c1.enter_context(tc.tile_pool(name="p1s", bufs=2))
            p1ps = c1.enter_context(tc.tile_pool(name="p1ps", bufs=1, space="PSUM"))

            wq_sb = p1w.tile([128, NKT, CPC], F32R, tag="wq")
            wk_sb = p1w.tile([128, NKT, CPC], F32R, tag="wk")
            wv_sb = p1w.tile([128, NKT, CPC], F32R, tag="wv")
            nc.sync.dma_start(
                wq_sb[:], wq[:].rearrange("(kt p) c -> p kt c", p=128).bitcast(F32R)
            )
            nc.sync.dma_start(
                wk_sb[:], wk[:].rearrange("(kt p) c -> p kt c", p=128).bitcast(F32R)
            )
            nc.sync.dma_start(
                wv_sb[:], wv[:].rearrange("(kt p) c -> p kt c", p=128).bitcast(F32R)
            )

            for ch in range(NCH):
                sl = slice(ch * 512, (ch + 1) * 512)
                ps_q = [
                    p1ps.tile([128, 512], F32, tag=f"psq{j}", name=f"psq{j}")
                    for j in range(HPC)
                ]
                ps_k = [
                    p1ps.tile([128, 512], F32, tag=f"psk{j}", name=f"psk{j}")
                    for j in range(HPC)
                ]
                ps_v = [
                    p1ps.tile([128, 256], F32, tag=f"psv{j}", name=f"psv{j}")
                    for j in range(4)
                ]
                for kt in range(NKT):
                    xc = p1s.tile([128, 512], F32R, tag="xc", bufs=3)
                    nc.sync.dma_start(
                        xc[:], xT[kt * 128 : (kt + 1) * 128, sl].bitcast(F32R)
                    )
                    st, sp = kt == 0, kt == NKT - 1
                    for j in range(HPC):
                        nc.tensor.matmul(
                            ps_q[j][:],
                            lhsT=wq_sb[:, kt, j * 128 : (j + 1) * 128],
                            rhs=xc[:],
                            start=st,
                            stop=sp,
                        )
                        nc.tensor.matmul(
                            ps_k[j][:],
                            lhsT=wk_sb[:, kt, j * 128 : (j + 1) * 128],
                            rhs=xc[:],
                            start=st,
                            stop=sp,
                        )
                    for j in range(4):
                        nc.tensor.matmul(
                            ps_v[j][:],
                            lhsT=xc[:, j * 128 : (j + 1) * 128],
                            rhs=wv_sb[:, kt, :],
                            start=st,
                            stop=sp,
                        )
                # v evacuation: add bias, store f32r
                for j in range(4):
                    nc.vector.tensor_tensor(
                        v_sb[:, ch * 4 + j, :],
                        ps_v[j][:],
                        bvb_sb[:],
                        op=mybir.AluOpType.add,
                    )
                # q/k evacuation + RoPE (rotate-half layout)
                cosc = p1s.tile([128, 512], F32, tag="cos")
                sinc = p1s.tile([128, 512], F32, tag="sin")
                nc.sync.dma_start(cosc[:], cosb[:, sl])
                nc.sync.dma_start(sinc[:], sinb[:, sl])
                for dst, ps_x, b_sb in ((qT, ps_q, bq_sb), (kT, ps_k, bk_sb)):
                    for j in range(HPC):
                        raw = p1s.tile([128, 512], F32, tag=f"raw{j}", bufs=2)
                        nc.scalar.activation(
                            raw[:], ps_x[j][:], IDENT, bias=b_sb[:, j : j + 1]
                        )
                        t1 = p1s.tile([128, 512], F32, tag=f"t1_{j}", bufs=2)
                        nc.vector.tensor_tensor(
                            t1[:], raw[:], cosc[:], op=mybir.AluOpType.mult
                        )
                        rot = p1s.tile([128, 512], F32, tag=f"rot{j}", bufs=2)
                        nc.vector.tensor_copy(rot[0:64, :], raw[64:128, :])
                        nc.vector.tensor_copy(rot[64:128, :], raw[0:64, :])
                        t2 = p1s.tile([128, 512], F32, tag=f"t2_{j}", bufs=2)
                        nc.vector.tensor_tensor(
                            t2[:], rot[:], sinc[:], op=mybir.AluOpType.mult
                        )
                        nc.vector.tensor_tensor(
                            dst[j][:, sl], t1[:], t2[:], op=mybir.AluOpType.add
                        )

        if DEBUG:
            for h in range(HPC):
                nc.sync.dma_start(
                    dbg_q[h * 128 : (h + 1) * 128, :], qT[h][:].bitcast(F32)
                )
                nc.sync.dma_start(
                    dbg_k[h * 128 : (h + 1) * 128, :], kT[h][:].bitcast(F32)
                )
            nc.sync.dma_start(
                dbg_v[:], v_sb[:].bitcast(F32).rearrange("p n c -> p (n c)")
            )

        # ---------------- Phase 2: attention per (batch, head) ----------------
        with ExitStack() as c2:
            p2s = c2.enter_context(tc.tile_pool(name="p2s", bufs=1))
            p2ps = c2.enter_context(tc.tile_pool(name="p2ps", bufs=1, space="PSUM"))

            mask_sb = p2s.tile([128, 2048], F32, tag="mask")
            nc.sync.dma_start(mask_sb[:], maskd[:])

            for b in range(B):
                for h in range(HPC):
                    qTh = qT[h][:, b * T : (b + 1) * T]
                    kTh = kT[h][:, b * T : (b + 1) * T]
                    for half in range(2):
                        # q-chunks covered by this half-pass
                        qcs = (half * 2, half * 2 + 1)
                        ps_out = p2ps.tile([128, 1024], F32, tag="pvout")
                        acc = p2s.tile([128, 1024], F32R, tag="dacc")
                        for kt in range(TKT):
                            sc = p2ps.tile([128, 1024], F32, tag="sc", bufs=2)
                            for i, qc in enumerate(qcs):
                                nc.tensor.matmul(
                                    sc[:, i * 512 : (i + 1) * 512],
                                    lhsT=kTh[:, kt * 128 : (kt + 1) * 128],
                                    rhs=qTh[:, qc * 512 : (qc + 1) * 512],
                                    start=True,
                                    stop=True,
                                )
                            et = p2s.tile([128, 1024], F32R, tag="et", bufs=3)
                            nc.scalar.activation(et[:], sc[:], EXP, scale=SM_SCALE)
                            if kt == 0:
                                nc.vector.tensor_copy(acc[:], et[:].bitcast(F32))
                            else:
                                nc.vector.tensor_tensor(
                                    acc[:],
                                    acc[:].bitcast(F32),
                                    et[:].bitcast(F32),
                                    op=mybir.AluOpType.add,
                                )
                            # PV accumulation (causal: kt-tile <= q-chunk)
                            vt = v_sb[:, b * TKT + kt, h * 128 : (h + 1) * 128]
                            ckt = kt // 4  # q-chunk containing this key tile
                            for i, qc in enumerate(qcs):
                                if ckt > qc:
                                    continue
                                st = kt == 0
                                sp = kt == 4 * qc + 3
                                if ckt == qc:
                                    a = kt % 4
                                    md = p2s.tile(
                                        [128, 512], F32R, tag="md", name="md", bufs=2
                                    )
                                    nc.vector.tensor_tensor(
                                        md[:],
                                        et[:, i * 512 : (i + 1) * 512].bitcast(F32),
                                        mask_sb[:, a * 512 : (a + 1) * 512],
                                        op=mybir.AluOpType.mult,
                                    )
                                    rhs = md[:]
                                else:
                                    rhs = et[:, i * 512 : (i + 1) * 512]
                                nc.tensor.matmul(
                                    ps_out[:, i * 512 : (i + 1) * 512],
                                    lhsT=vt,
                                    rhs=rhs,
                                    start=st,
                                    stop=sp,
                                )
                        # denominator: sum over the 128 partitions of acc
                        dsum = p2ps.tile([1, 1024], F32, tag="dsum")
                        for i in range(2):
                            nc.tensor.matmul(
                                dsum[:, i * 512 : (i + 1) * 512],
                                lhsT=ones_sb[:],
                                rhs=acc[:, i * 512 : (i + 1) * 512],
                                start=True,
                                stop=True,
                            )
                        dsb = p2s.tile([1, 1024], F32, tag="dsb")
                        nc.scalar.copy(dsb[:], dsum[:])
                        rsb = p2s.tile([1, 1024], F32, tag="rsb")
                        nc.vector.reciprocal(rsb[:], dsb[:])
                        rbc = p2s.tile([128, 1024], F32, tag="rbc")
                        nc.gpsimd.partition_broadcast(rbc[:], rsb[:])
                        # y^T = out^T * (1/denom), store to DRAM
                        yt = p2s.tile([128, 1024], F32, tag="yt", bufs=2)
                        nc.vector.tensor_tensor(
                            yt[:], ps_out[:], rbc[:], op=mybir.AluOpType.mult
                        )
                        nc.sync.dma_start(
                            yT_local[
                                h * 128 : (h + 1) * 128,
                                b * T + half * 1024 : b * T + (half + 1) * 1024,
                            ],
                            yt[:],
                        )

        # ---------------- Phase 3: AllGather + output projection ----------------
        if DEBUG:
            nc.sync.dma_start(dbg_y[:], yT_local[:])
        nc.gpsimd.collective_compute(
            "AllGather",
            mybir.AluOpType.bypass,
            replica_groups=[list(range(NCORES))],
            ins=[yT_local[:]],
            outs=[yT_all[:]],
        )
        with ExitStack() as c3:
            p3s = c3.enter_context(tc.tile_pool(name="p3s", bufs=1))
            p3ps = c3.enter_context(tc.tile_pool(name="p3ps", bufs=1, space="PSUM"))

            wo_sb = p3s.tile([128, NKT, CPC], F32R, tag="wo")
            nc.sync.dma_start(
                wo_sb[:], wo[:].rearrange("(kt p) c -> p kt c", p=128).bitcast(F32R)
            )
            for ch in range(NCH):
                sl = slice(ch * 512, (ch + 1) * 512)
                ps_o = [
                    p3ps.tile([128, 512], F32, tag=f"pso{j}", name=f"pso{j}", bufs=2)
                    for j in range(HPC)
                ]
                for kt in range(NKT):
                    yp = p3s.tile([128, 512], F32R, tag="yp", bufs=4)
                    nc.sync.dma_start(
                        yp[:], yT_all[kt * 128 : (kt + 1) * 128, sl].bitcast(F32R)
                    )
                    for j in range(HPC):
                        nc.tensor.matmul(
                            ps_o[j][:],
                            lhsT=wo_sb[:, kt, j * 128 : (j + 1) * 128],
                            rhs=yp[:],
                            start=kt == 0,
                            stop=kt == NKT - 1,
                        )
                for j in range(HPC):
                    ot = p3s.tile([128, 512], F32, tag=f"ot{j}", bufs=2)
                    nc.scalar.activation(
                        ot[:], ps_o[j][:], IDENT, bias=bo_sb[:, j : j + 1]
                    )
                    nc.sync.dma_start(oT[j * 128 : (j + 1) * 128, sl], ot[:])

    nc.compile()
    return nc


def _build():
    global _NC_CACHE
    if _NC_CACHE is None:
        nc = bacc.Bacc("TRN2", target_bir_lowering=False)
        _NC_CACHE = _emit(nc)
    return _NC_CACHE


def _host_prep(x, Wq, bq, Wk, bk, Wv, bv, Wo, bo):
    """Build the 8 per-core input maps."""
    xT = np.ascontiguousarray(x.reshape(BT, H).T)

    # de-interleave rope pairs: new[d] = old[2d], new[64+d] = old[2d+1]
    perm = np.concatenate([np.arange(0, HS, 2), np.arange(1, HS, 2)])

    t = np.arange(T, dtype=np.float64)
    invf = 1.0 / (10000.0 ** (np.arange(0, HS, 2, dtype=np.float64) / HS))
    ang = invf[:, None] * t[None, :]  # (64, T)
    cos64 = np.cos(ang)
    sin64 = np.sin(ang)
    cosb = np.tile(
        np.concatenate([cos64, cos64], axis=0).astype(np.float32), (1, B)
    )
    sinb = np.tile(
        np.concatenate([-sin64, sin64], axis=0).astype(np.float32), (1, B)
    )

    maskd = np.zeros((128, 2048), dtype=np.float32)
    p = np.arange(128)[:, None]
    j = np.arange(512)[None, :]
    for a in range(4):
        maskd[:, a * 512 : (a + 1) * 512] = (p + a * 128 <= j).astype(np.float32)

    in_maps = []
    for c in range(NCORES):
        heads = [HPC * c + i for i in range(HPC)]
        wq_c = np.concatenate(
            [Wq[:, h * HS : (h + 1) * HS][:, perm] for h in heads], axis=1
        )
        wk_c = np.concatenate(
            [Wk[:, h * HS : (h + 1) * HS][:, perm] for h in heads], axis=1
        )
        wv_c = np.concatenate([Wv[:, h * HS : (h + 1) * HS] for h in heads], axis=1)
        bq_c = np.stack([bq[h * HS : (h + 1) * HS][perm] for h in heads], axis=1)
        bk_c = np.stack([bk[h * HS : (h + 1) * HS][perm] for h in heads], axis=1)
        bv_c = np.concatenate([bv[h * HS : (h + 1) * HS] for h in heads])
        cols = slice(c * CPC, (c + 1) * CPC)
        wo_c = Wo[:, cols]
        bo_c = np.stack(
            [bo[c * CPC + i * HS : c * CPC + (i + 1) * HS] for i in range(HPC)],
            axis=1,
        )
        in_maps.append(
            {
                "xT": xT,
                "wq": np.ascontiguousarray(wq_c),
                "wk": np.ascontiguousarray(wk_c),
                "wv": np.ascontiguousarray(wv_c),
                "bq2": np.ascontiguousarray(bq_c),
                "bk2": np.ascontiguousarray(bk_c),
                "bvb": np.broadcast_to(bv_c, (128, CPC)).copy(),
                "wo": np.ascontiguousarray(wo_c),
                "bo2": np.ascontiguousarray(bo_c),
                "cosb": cosb,
                "sinb": sinb,
                "maskd": maskd,
                "onesc": np.ones((128, 1), dtype=np.float32),
            }
        )
    return in_maps


def run_sharded(inputs, trace=False):
    """Run the SPMD kernel; returns (output (B,T,H), BassKernelResults)."""
    in_maps = _host_prep(
        np.asarray(inputs["x"], dtype=np.float32),
        np.asarray(inputs["Wq"], dtype=np.float32),
        np.asarray(inputs["bq"], dtype=np.float32),
        np.asarray(inputs["Wk"], dtype=np.float32),
        np.asarray(inputs["bk"], dtype=np.float32),
        np.asarray(inputs["Wv"], dtype=np.float32),
        np.asarray(inputs["bv"], dtype=np.float32),
        np.asarray(inputs["Wo"], dtype=np.float32),
        np.asarray(inputs["bo"], dtype=np.float32),
    )
    nc = _build()
    res = run_bass_kernel_spmd(nc, in_maps, core_ids=list(range(NCORES)), trace=trace)
    o = np.empty((BT, H), dtype=np.float32)
    for c in range(NCORES):
        o[:, c * CPC : (c + 1) * CPC] = res.results[c]["oT"].T
    return o.reshape(B, T, H), res


def kernel(**inputs):
    out, _ = run_sharded(inputs, trace=False)
    return out


# revision 20
# speedup vs baseline: 1.1320x; 1.1320x over previous
"""Causal self-attention (B=2, T=2048, H=2048, NH=16) on 8 TRN2 NeuronCores.

Sharding: head-parallel. Core c owns heads 2c, 2c+1 (256 of 2048 qkv columns)
for both batch elements, computes its heads' attention output yT, AllGathers
yT across cores (one collective per (batch, head) instance, overlapped with
compute of later instances), then computes a 256-column slice of the output
projection.

Layout choices (all matmuls transpose-free):
  - x is passed pre-transposed (xT: [H, B*T]) so projections contract
    feature-on-partition.
  - q/k are produced transposed ([hs, tok]) and RoPE is applied in
    rotate-half form: Wq/Wk columns are de-interleaved per head on the host.
  - scores are computed transposed (s^T[kt, q]) so the PV matmul and the
    output projection need no on-device transposes.
  - softmax denominator: ones-vector matmuls on the PE accumulate column
    sums of exp(s^T) in PSUM; reciprocal broadcast via gpsimd.
  - mask-after-softmax (no renorm) per the reference: full-row softmax, then
    tril mask; strictly-above-diagonal PV blocks are skipped entirely.
"""

import sys

sys.path.insert(0, "/opt/trn_rl_repo")

import numpy as np
from contextlib import ExitStack

import concourse.mybir as mybir
import concourse.tile as tile
from concourse import bacc
from concourse.bass_utils import run_bass_kernel_spmd

B, T, H, NH, HS = 2, 2048, 2048, 16, 128
NCORES = 8
HPC = NH // NCORES        # heads per core: 2
CPC = HPC * HS            # qkv columns per core: 256
BT = B * T                # 4096
NKT = H // 128            # contraction k-tiles: 16
TKT = T // 128            # key tiles per batch: 16
NCH = BT // 512           # token 512-chunks: 8
F32 = mybir.dt.float32
F32R = mybir.dt.float32r
EXP = mybir.ActivationFunctionType.Exp
IDENT = mybir.ActivationFunctionType.Identity
SM_SCALE = float(1.0 / np.sqrt(HS))

_NC_CACHE = None
DEBUG = False


def _emit(nc):
    xT = nc.dram_tensor("xT", [H, BT], F32, kind="ExternalInput")
    wq = nc.dram_tensor("wq", [H, CPC], F32, kind="ExternalInput")
    wk = nc.dram_tensor("wk", [H, CPC], F32, kind="ExternalInput")
    wv = nc.dram_tensor("wv", [H, CPC], F32, kind="ExternalInput")
    bq2 = nc.dram_tensor("bq2", [128, HPC], F32, kind="ExternalInput")
    bk2 = nc.dram_tensor("bk2", [128, HPC], F32, kind="ExternalInput")
    bvb = nc.dram_tensor("bvb", [128, CPC], F32, kind="ExternalInput")
    wo = nc.dram_tensor("wo", [H, CPC], F32, kind="ExternalInput")
    bo2 = nc.dram_tensor("bo2", [128, HPC], F32, kind="ExternalInput")
    cosb = nc.dram_tensor("cosb", [128, BT], F32, kind="ExternalInput")
    sinb = nc.dram_tensor("sinb", [128, BT], F32, kind="ExternalInput")
    maskd = nc.dram_tensor("maskd", [128, 2048], F32, kind="ExternalInput")
    onesc = nc.dram_tensor("onesc", [128, 1], F32, kind="ExternalInput")
    oT = nc.dram_tensor("oT", [CPC, BT], F32, kind="ExternalOutput")

    # per-(batch, head) gather buffers
    yg_in = [
        [nc.dram_tensor(f"yg_in_{b}_{h}", [128, T], F32) for h in range(HPC)]
        for b in range(B)
    ]
    yg_all = [
        [
            nc.dram_tensor(
                f"yg_all_{b}_{h}", [NCORES * 128, T], F32, addr_space="Shared"
            )
            for h in range(HPC)
        ]
        for b in range(B)
    ]
    if DEBUG:
        dbg_q = nc.dram_tensor("dbg_q", [CPC, BT], F32, kind="ExternalOutput")
        dbg_k = nc.dram_tensor("dbg_k", [CPC, BT], F32, kind="ExternalOutput")
        dbg_v = nc.dram_tensor(
            "dbg_v", [128, (BT // 128) * CPC], F32, kind="ExternalOutput"
        )
        dbg_y = nc.dram_tensor("dbg_y", [CPC, BT], F32, kind="ExternalOutput")

    with tile.TileContext(nc) as tc, ExitStack() as ctx:
        persist = ctx.enter_context(tc.tile_pool(name="persist", bufs=1))

        qT = [
            persist.tile([128, BT], F32R, tag=f"qT{h}", name=f"qT{h}")
            for h in range(HPC)
        ]
        kT = [
            persist.tile([128, BT], F32R, tag=f"kT{h}", name=f"kT{h}")
            for h in range(HPC)
        ]
        v_sb = persist.tile([128, BT // 128, CPC], F32R, tag="v")
        bq_sb = persist.tile([128, HPC], F32, tag="bq")
        bk_sb = persist.tile([128, HPC], F32, tag="bk")
        bo_sb = persist.tile([128, HPC], F32, tag="bo")
        bvb_sb = persist.tile([128, CPC], F32, tag="bvb")
        ones_sb = persist.tile([128, 1], F32R, tag="ones")
        nc.sync.dma_start(bq_sb[:], bq2[:])
        nc.sync.dma_start(bk_sb[:], bk2[:])
        nc.sync.dma_start(bo_sb[:], bo2[:])
        nc.sync.dma_start(bvb_sb[:], bvb[:])
        nc.sync.dma_start(ones_sb[:], onesc[:].bitcast(F32R))

        # ---------------- Phase 1: QKV projections + RoPE ----------------
        with ExitStack() as c1:
            p1w = c1.enter_context(tc.tile_pool(name="p1w", bufs=1))
            p1s = c1.enter_context(tc.tile_pool(name="p1s", bufs=2))
            p1ps = c1.enter_context(tc.tile_pool(name="p1ps", bufs=1, space="PSUM"))

            wq_sb = p1w.tile([128, NKT, CPC], F32R, tag="wq")
            wk_sb = p1w.tile([128, NKT, CPC], F32R, tag="wk")
            wv_sb = p1w.tile([128, NKT, CPC], F32R, tag="wv")
            nc.sync.dma_start(
                wq_sb[:], wq[:].rearrange("(kt p) c -> p kt c", p=128).bitcast(F32R)
            )
            nc.sync.dma_start(
                wk_sb[:], wk[:].rearrange("(kt p) c -> p kt c", p=128).bitcast(F32R)
            )
            nc.sync.dma_start(
                wv_sb[:], wv[:].rearrange("(kt p) c -> p kt c", p=128).bitcast(F32R)
            )

            for ch in range(NCH):
                sl = slice(ch * 512, (ch + 1) * 512)
                ps_q = [
                    p1ps.tile([128, 512], F32, tag=f"psq{j}", name=f"psq{j}")
                    for j in range(HPC)
                ]
                ps_k = [
                    p1ps.tile([128, 512], F32, tag=f"psk{j}", name=f"psk{j}")
                    for j in range(HPC)
                ]
                ps_v = [
                    p1ps.tile([128, 256], F32, tag=f"psv{j}", name=f"psv{j}")
                    for j in range(4)
                ]
                for kt in range(NKT):
                    xc = p1s.tile([128, 512], F32R, tag="xc", bufs=3)
                    nc.sync.dma_start(
                        xc[:], xT[kt * 128 : (kt + 1) * 128, sl].bitcast(F32R)
                    )
                    st, sp = kt == 0, kt == NKT - 1
                    for j in range(HPC):
                        nc.tensor.matmul(
                            ps_q[j][:],
                            lhsT=wq_sb[:, kt, j * 128 : (j + 1) * 128],
                            rhs=xc[:],
                            start=st,
                            stop=sp,
                        )
                        nc.tensor.matmul(
                            ps_k[j][:],
                            lhsT=wk_sb[:, kt, j * 128 : (j + 1) * 128],
                            rhs=xc[:],
                            start=st,
                            stop=sp,
                        )
                    for j in range(4):
                        nc.tensor.matmul(
                            ps_v[j][:],
                            lhsT=xc[:, j * 128 : (j + 1) * 128],
                            rhs=wv_sb[:, kt, :],
                            start=st,
                            stop=sp,
                        )
                for j in range(4):
                    nc.vector.tensor_tensor(
                        v_sb[:, ch * 4 + j, :],
                        ps_v[j][:],
                        bvb_sb[:],
                        op=mybir.AluOpType.add,
                    )
                # q/k evacuation + RoPE (rotate-half via partition-offset reads)
                cosc = p1s.tile([128, 512], F32, tag="cos")
                sinc = p1s.tile([128, 512], F32, tag="sin")
                nc.sync.dma_start(cosc[:], cosb[:, sl])
                nc.sync.dma_start(sinc[:], sinb[:, sl])
                for dst, ps_x, b_sb in ((qT, ps_q, bq_sb), (kT, ps_k, bk_sb)):
                    for j in range(HPC):
                        raw = p1s.tile([128, 512], F32, tag=f"raw{j}", bufs=2)
                        nc.scalar.activation(
                            raw[:], ps_x[j][:], IDENT, bias=b_sb[:, j : j + 1]
                        )
                        t1 = p1s.tile([128, 512], F32, tag=f"t1_{j}", bufs=2)
                        nc.vector.tensor_tensor(
                            t1[:], raw[:], cosc[:], op=mybir.AluOpType.mult
                        )
                        rot = p1s.tile([128, 512], F32, tag=f"rot{j}", bufs=2)
                        nc.vector.tensor_copy(rot[0:64, :], raw[64:128, :])
                        nc.vector.tensor_copy(rot[64:128, :], raw[0:64, :])
                        t2 = p1s.tile([128, 512], F32, tag=f"t2_{j}", bufs=2)
                        nc.vector.tensor_tensor(
                            t2[:], rot[:], sinc[:], op=mybir.AluOpType.mult
                        )
                        nc.vector.tensor_tensor(
                            dst[j][:, sl], t1[:], t2[:], op=mybir.AluOpType.add
                        )

        if DEBUG:
            for h in range(HPC):
                nc.sync.dma_start(
                    dbg_q[h * 128 : (h + 1) * 128, :], qT[h][:].bitcast(F32)
                )
                nc.sync.dma_start(
                    dbg_k[h * 128 : (h + 1) * 128, :], kT[h][:].bitcast(F32)
                )
            nc.sync.dma_start(
                dbg_v[:], v_sb[:].bitcast(F32).rearrange("p n c -> p (n c)")
            )

        # -------- Phase 2: attention per (batch, head) + pipelined gathers ----
        # -------- Phase 3: output projection (SBUF shared pool; PSUM separate)
        with ExitStack() as c2:
            p2s = c2.enter_context(tc.tile_pool(name="p2s", bufs=1))
            c2ps = ExitStack()
            p2ps = c2ps.enter_context(tc.tile_pool(name="p2ps", bufs=1, space="PSUM"))

            mask_sb = p2s.tile([128, 2048], F32, tag="mask")
            nc.sync.dma_start(mask_sb[:], maskd[:])
            # prefetch Wo during phase 2
            wo_sb = p2s.tile([128, NKT, CPC], F32R, tag="wo")
            nc.sync.dma_start(
                wo_sb[:], wo[:].rearrange("(kt p) c -> p kt c", p=128).bitcast(F32R)
            )

            for b in range(B):
                for h in range(HPC):
                    qTh = qT[h][:, b * T : (b + 1) * T]
                    kTh = kT[h][:, b * T : (b + 1) * T]
                    for half in range(2):
                        qcs = (half * 2, half * 2 + 1)
                        ps_out = p2ps.tile([128, 1024], F32, tag="pvout")
                        dsum = p2ps.tile([1, 1024], F32, tag="dsum")
                        for kt in range(TKT):
                            sc = p2ps.tile([128, 1024], F32, tag="sc", bufs=2)
                            for i, qc in enumerate(qcs):
                                nc.tensor.matmul(
                                    sc[:, i * 512 : (i + 1) * 512],
                                    lhsT=kTh[:, kt * 128 : (kt + 1) * 128],
                                    rhs=qTh[:, qc * 512 : (qc + 1) * 512],
                                    start=True,
                                    stop=True,
                                )
                            et = p2s.tile([128, 1024], F32R, tag="et", bufs=3)
                            nc.scalar.activation(et[:], sc[:], EXP, scale=SM_SCALE)
                            # denominator accumulation on PE
                            for i in range(2):
                                nc.tensor.matmul(
                                    dsum[:, i * 512 : (i + 1) * 512],
                                    lhsT=ones_sb[:],
                                    rhs=et[:, i * 512 : (i + 1) * 512],
                                    start=kt == 0,
                                    stop=kt == TKT - 1,
                                )
                            # PV accumulation (causal: key tile <= q-chunk)
                            vt = v_sb[:, b * TKT + kt, h * 128 : (h + 1) * 128]
                            ckt = kt // 4
                            for i, qc in enumerate(qcs):
                                if ckt > qc:
                                    continue
                                st = kt == 0
                                sp = kt == 4 * qc + 3
                                if ckt == qc:
                                    a = kt % 4
                                    md = p2s.tile(
                                        [128, 512], F32R, tag="md", name="md", bufs=2
                                    )
                                    nc.vector.tensor_tensor(
                                        md[:],
                                        et[:, i * 512 : (i + 1) * 512].bitcast(F32),
                                        mask_sb[:, a * 512 : (a + 1) * 512],
                                        op=mybir.AluOpType.mult,
                                    )
                                    rhs = md[:]
                                else:
                                    rhs = et[:, i * 512 : (i + 1) * 512]
                                nc.tensor.matmul(
                                    ps_out[:, i * 512 : (i + 1) * 512],
                                    lhsT=vt,
                                    rhs=rhs,
                                    start=st,
                                    stop=sp,
                                )
                        dsb = p2s.tile([1, 1024], F32, tag="dsb")
                        nc.scalar.copy(dsb[:], dsum[:])
                        rsb = p2s.tile([1, 1024], F32, tag="rsb")
                        nc.vector.reciprocal(rsb[:], dsb[:])
                        rbc = p2s.tile([128, 1024], F32, tag="rbc")
                        nc.gpsimd.partition_broadcast(rbc[:], rsb[:])
                        yt = p2s.tile([128, 1024], F32, tag="yt", bufs=2)
                        nc.vector.tensor_tensor(
                            yt[:], ps_out[:], rbc[:], op=mybir.AluOpType.mult
                        )
                        nc.sync.dma_start(
                            yg_in[b][h][:, half * 1024 : (half + 1) * 1024], yt[:]
                        )
                    # gather this instance across cores (overlaps later compute)
                    nc.gpsimd.collective_compute(
                        "AllGather",
                        mybir.AluOpType.bypass,
                        replica_groups=[list(range(NCORES))],
                        ins=[yg_in[b][h][:]],
                        outs=[yg_all[b][h][:]],
                    )
                    if DEBUG:
                        nc.sync.dma_start(
                            dbg_y[h * 128 : (h + 1) * 128, b * T : (b + 1) * T],
                            yg_in[b][h][:],
                        )

            # ---------------- Phase 3: output projection ----------------
            c2ps.close()
            with ExitStack() as c3:
                p3ps = c3.enter_context(
                    tc.tile_pool(name="p3ps", bufs=1, space="PSUM")
                )
                for b in range(B):
                    for cq in range(T // 512):
                        sl_t = slice(cq * 512, (cq + 1) * 512)
                        ps_o = [
                            p3ps.tile(
                                [128, 512], F32, tag=f"pso{j}", name=f"pso{j}", bufs=2
                            )
                            for j in range(HPC)
                        ]
                        for kt in range(NKT):
                            src = yg_all[b][kt % 2]
                            yp = p2s.tile([128, 512], F32R, tag="yp", bufs=4)
                            nc.sync.dma_start(
                                yp[:],
                                src[
                                    (kt // 2) * 128 : (kt // 2 + 1) * 128, sl_t
                                ].bitcast(F32R),
                            )
                            for j in range(HPC):
                                nc.tensor.matmul(
                                    ps_o[j][:],
                                    lhsT=wo_sb[:, kt, j * 128 : (j + 1) * 128],
                                    rhs=yp[:],
                                    start=kt == 0,
                                    stop=kt == NKT - 1,
                                )
                        for j in range(HPC):
                            ot = p2s.tile(
                                [128, 512], F32, tag=f"ot{j}", name=f"ot{j}", bufs=2
                            )
                            nc.scalar.activation(
                                ot[:], ps_o[j][:], IDENT, bias=bo_sb[:, j : j + 1]
                            )
                            nc.sync.dma_start(
                                oT[j * 128 : (j + 1) * 128, b * T + cq * 512 : b * T + (cq + 1) * 512],
                                ot[:],
                            )

    nc.compile()
    return nc


def _build():
    global _NC_CACHE
    if _NC_CACHE is None:
        nc = bacc.Bacc("TRN2", target_bir_lowering=False)
        _NC_CACHE = _emit(nc)
    return _NC_CACHE


def _host_prep(x, Wq, bq, Wk, bk, Wv, bv, Wo, bo):
    """Build the 8 per-core input maps."""
    xT = np.ascontiguousarray(x.reshape(BT, H).T)

    # de-interleave rope pairs: new[d] = old[2d], new[64+d] = old[2d+1]
    perm = np.concatenate([np.arange(0, HS, 2), np.arange(1, HS, 2)])

    t = np.arange(T, dtype=np.float64)
    invf = 1.0 / (10000.0 ** (np.arange(0, HS, 2, dtype=np.float64) / HS))
    ang = invf[:, None] * t[None, :]  # (64, T)
    cos64 = np.cos(ang)
    sin64 = np.sin(ang)
    cosb = np.tile(np.concatenate([cos64, cos64], axis=0).astype(np.float32), (1, B))
    sinb = np.tile(np.concatenate([-sin64, sin64], axis=0).astype(np.float32), (1, B))

    maskd = np.zeros((128, 2048), dtype=np.float32)
    p = np.arange(128)[:, None]
    j = np.arange(512)[None, :]
    for a in range(4):
        maskd[:, a * 512 : (a + 1) * 512] = (p + a * 128 <= j).astype(np.float32)

    in_maps = []
    for c in range(NCORES):
        heads = [HPC * c + i for i in range(HPC)]
        wq_c = np.concatenate(
            [Wq[:, h * HS : (h + 1) * HS][:, perm] for h in heads], axis=1
        )
        wk_c = np.concatenate(
            [Wk[:, h * HS : (h + 1) * HS][:, perm] for h in heads], axis=1
        )
        wv_c = np.concatenate([Wv[:, h * HS : (h + 1) * HS] for h in heads], axis=1)
        bq_c = np.stack([bq[h * HS : (h + 1) * HS][perm] for h in heads], axis=1)
        bk_c = np.stack([bk[h * HS : (h + 1) * HS][perm] for h in heads], axis=1)
        bv_c = np.concatenate([bv[h * HS : (h + 1) * HS] for h in heads])
        cols = slice(c * CPC, (c + 1) * CPC)
        wo_c = Wo[:, cols]
        bo_c = np.stack(
            [bo[c * CPC + i * HS : c * CPC + (i + 1) * HS] for i in range(HPC)],
            axis=1,
        )
        in_maps.append(
            {
                "xT": xT,
                "wq": np.ascontiguousarray(wq_c),
                "wk": np.ascontiguousarray(wk_c),
                "wv": np.ascontiguousarray(wv_c),
                "bq2": np.ascontiguousarray(bq_c),
                "bk2": np.ascontiguousarray(bk_c),
                "bvb": np.broadcast_to(bv_c, (128, CPC)).copy(),
                "wo": np.ascontiguousarray(wo_c),
                "bo2": np.ascontiguousarray(bo_c),
                "cosb": cosb,
                "sinb": sinb,
                "maskd": maskd,
                "onesc": np.ones((128, 1), dtype=np.float32),
            }
        )
    return in_maps


def run_sharded(inputs, trace=False):
    """Run the SPMD kernel; returns (output (B,T,H), BassKernelResults)."""
    in_maps = _host_prep(
        np.asarray(inputs["x"], dtype=np.float32),
        np.asarray(inputs["Wq"], dtype=np.float32),
        np.asarray(inputs["bq"], dtype=np.float32),
        np.asarray(inputs["Wk"], dtype=np.float32),
        np.asarray(inputs["bk"], dtype=np.float32),
        np.asarray(inputs["Wv"], dtype=np.float32),
        np.asarray(inputs["bv"], dtype=np.float32),
        np.asarray(inputs["Wo"], dtype=np.float32),
        np.asarray(inputs["bo"], dtype=np.float32),
    )
    nc = _build()
    res = run_bass_kernel_spmd(nc, in_maps, core_ids=list(range(NCORES)), trace=trace)
    o = np.empty((BT, H), dtype=np.float32)
    for c in range(NCORES):
        o[:, c * CPC : (c + 1) * CPC] = res.results[c]["oT"].T
    return o.reshape(B, T, H), res


def kernel(**inputs):
    out, _ = run_sharded(inputs, trace=False)
    return out


# revision 21
# speedup vs baseline: 1.2880x; 1.1379x over previous
"""Causal self-attention (B=2, T=2048, H=2048, NH=16) on 8 TRN2 NeuronCores.

Sharding: head-parallel. Core c owns heads 2c, 2c+1 (256 of 2048 qkv columns)
for both batch elements, computes its heads' attention output yT, AllGathers
yT across cores (one collective per (batch, head) instance, overlapped with
compute of later instances), then computes a 256-column slice of the output
projection.

Layout choices (all matmuls transpose-free):
  - x is passed pre-transposed (xT: [H, B*T]) so projections contract
    feature-on-partition.
  - q/k are produced transposed ([hs, tok]) and RoPE is applied in
    rotate-half form: Wq/Wk columns are de-interleaved per head on the host.
  - scores are computed transposed (s^T[kt, q]) so the PV matmul and the
    output projection need no on-device transposes.
  - softmax denominator: ones-vector matmuls on the PE accumulate column
    sums of exp(s^T) in PSUM; reciprocal broadcast via gpsimd.
  - mask-after-softmax (no renorm) per the reference: full-row softmax, then
    tril mask; strictly-above-diagonal PV blocks are skipped entirely.
"""

import sys

sys.path.insert(0, "/opt/trn_rl_repo")

import numpy as np
from contextlib import ExitStack

import concourse.mybir as mybir
import concourse.tile as tile
from concourse import bacc
from concourse.bass_utils import run_bass_kernel_spmd

B, T, H, NH, HS = 2, 2048, 2048, 16, 128
NCORES = 8
HPC = NH // NCORES        # heads per core: 2
CPC = HPC * HS            # qkv columns per core: 256
BT = B * T                # 4096
NKT = H // 128            # contraction k-tiles: 16
TKT = T // 128            # key tiles per batch: 16
NCH = BT // 512           # token 512-chunks: 8
F32 = mybir.dt.float32
F32R = mybir.dt.float32r
EXP = mybir.ActivationFunctionType.Exp
IDENT = mybir.ActivationFunctionType.Identity
SM_SCALE = float(1.0 / np.sqrt(HS))

_NC_CACHE = None
DEBUG = False


def _emit(nc):
    xT = nc.dram_tensor("xT", [H, BT], F32, kind="ExternalInput")
    wq = nc.dram_tensor("wq", [H, CPC], F32, kind="ExternalInput")
    wk = nc.dram_tensor("wk", [H, CPC], F32, kind="ExternalInput")
    wv = nc.dram_tensor("wv", [H, CPC], F32, kind="ExternalInput")
    bq2 = nc.dram_tensor("bq2", [128, HPC], F32, kind="ExternalInput")
    bk2 = nc.dram_tensor("bk2", [128, HPC], F32, kind="ExternalInput")
    bvb = nc.dram_tensor("bvb", [128, CPC], F32, kind="ExternalInput")
    wo = nc.dram_tensor("wo", [H, CPC], F32, kind="ExternalInput")
    bo2 = nc.dram_tensor("bo2", [128, HPC], F32, kind="ExternalInput")
    cosb = nc.dram_tensor("cosb", [128, BT], F32, kind="ExternalInput")
    sinb = nc.dram_tensor("sinb", [128, BT], F32, kind="ExternalInput")
    maskd = nc.dram_tensor("maskd", [128, 2048], F32, kind="ExternalInput")
    onesc = nc.dram_tensor("onesc", [128, 1], F32, kind="ExternalInput")
    oT = nc.dram_tensor("oT", [CPC, BT], F32, kind="ExternalOutput")

    # per-(batch, head) gather buffers
    yg_in = [
        [nc.dram_tensor(f"yg_in_{b}_{h}", [128, T], F32) for h in range(HPC)]
        for b in range(B)
    ]
    yg_all = [
        [
            nc.dram_tensor(
                f"yg_all_{b}_{h}", [NCORES * 128, T], F32, addr_space="Shared"
            )
            for h in range(HPC)
        ]
        for b in range(B)
    ]
    if DEBUG:
        dbg_q = nc.dram_tensor("dbg_q", [CPC, BT], F32, kind="ExternalOutput")
        dbg_k = nc.dram_tensor("dbg_k", [CPC, BT], F32, kind="ExternalOutput")
        dbg_v = nc.dram_tensor(
            "dbg_v", [128, (BT // 128) * CPC], F32, kind="ExternalOutput"
        )
        dbg_y = nc.dram_tensor("dbg_y", [CPC, BT], F32, kind="ExternalOutput")

    with tile.TileContext(nc) as tc, ExitStack() as ctx:
        persist = ctx.enter_context(tc.tile_pool(name="persist", bufs=1))

        qT = [
            persist.tile([128, BT], F32R, tag=f"qT{h}", name=f"qT{h}")
            for h in range(HPC)
        ]
        kT = [
            persist.tile([128, BT], F32R, tag=f"kT{h}", name=f"kT{h}")
            for h in range(HPC)
        ]
        v_sb = persist.tile([128, BT // 128, CPC], F32R, tag="v")
        bq_sb = persist.tile([128, HPC], F32, tag="bq")
        bk_sb = persist.tile([128, HPC], F32, tag="bk")
        bo_sb = persist.tile([128, HPC], F32, tag="bo")
        bvb_sb = persist.tile([128, CPC], F32, tag="bvb")
        ones_sb = persist.tile([128, 1], F32R, tag="ones")
        nc.sync.dma_start(bq_sb[:], bq2[:])
        nc.sync.dma_start(bk_sb[:], bk2[:])
        nc.sync.dma_start(bo_sb[:], bo2[:])
        nc.sync.dma_start(bvb_sb[:], bvb[:])
        nc.sync.dma_start(ones_sb[:], onesc[:].bitcast(F32R))

        # ---------------- Phase 1: QKV projections + RoPE ----------------
        with ExitStack() as c1:
            p1w = c1.enter_context(tc.tile_pool(name="p1w", bufs=1))
            p1s = c1.enter_context(tc.tile_pool(name="p1s", bufs=2))
            p1ps = c1.enter_context(tc.tile_pool(name="p1ps", bufs=1, space="PSUM"))

            wq_sb = p1w.tile([128, NKT, CPC], F32R, tag="wq")
            wk_sb = p1w.tile([128, NKT, CPC], F32R, tag="wk")
            wv_sb = p1w.tile([128, NKT, CPC], F32R, tag="wv")
            nc.sync.dma_start(
                wq_sb[:], wq[:].rearrange("(kt p) c -> p kt c", p=128).bitcast(F32R)
            )
            nc.sync.dma_start(
                wk_sb[:], wk[:].rearrange("(kt p) c -> p kt c", p=128).bitcast(F32R)
            )
            nc.sync.dma_start(
                wv_sb[:], wv[:].rearrange("(kt p) c -> p kt c", p=128).bitcast(F32R)
            )

            for ch in range(NCH):
                sl = slice(ch * 512, (ch + 1) * 512)
                ps_q = [
                    p1ps.tile([128, 512], F32, tag=f"psq{j}", name=f"psq{j}")
                    for j in range(HPC)
                ]
                ps_k = [
                    p1ps.tile([128, 512], F32, tag=f"psk{j}", name=f"psk{j}")
                    for j in range(HPC)
                ]
                ps_v = [
                    p1ps.tile([128, 256], F32, tag=f"psv{j}", name=f"psv{j}")
                    for j in range(4)
                ]
                for kt in range(NKT):
                    xc = p1s.tile([128, 512], F32R, tag="xc", bufs=3)
                    nc.sync.dma_start(
                        xc[:], xT[kt * 128 : (kt + 1) * 128, sl].bitcast(F32R)
                    )
                    st, sp = kt == 0, kt == NKT - 1
                    for j in range(HPC):
                        nc.tensor.matmul(
                            ps_q[j][:],
                            lhsT=wq_sb[:, kt, j * 128 : (j + 1) * 128],
                            rhs=xc[:],
                            start=st,
                            stop=sp,
                        )
                        nc.tensor.matmul(
                            ps_k[j][:],
                            lhsT=wk_sb[:, kt, j * 128 : (j + 1) * 128],
                            rhs=xc[:],
                            start=st,
                            stop=sp,
                        )
                    for j in range(4):
                        nc.tensor.matmul(
                            ps_v[j][:],
                            lhsT=xc[:, j * 128 : (j + 1) * 128],
                            rhs=wv_sb[:, kt, :],
                            start=st,
                            stop=sp,
                        )
                for j in range(4):
                    nc.vector.tensor_tensor(
                        v_sb[:, ch * 4 + j, :],
                        ps_v[j][:],
                        bvb_sb[:],
                        op=mybir.AluOpType.add,
                    )
                # q/k evacuation + RoPE (rotate-half via partition-offset reads)
                cosc = p1s.tile([128, 512], F32, tag="cos")
                sinc = p1s.tile([128, 512], F32, tag="sin")
                nc.sync.dma_start(cosc[:], cosb[:, sl])
                nc.sync.dma_start(sinc[:], sinb[:, sl])
                for dst, ps_x, b_sb in ((qT, ps_q, bq_sb), (kT, ps_k, bk_sb)):
                    for j in range(HPC):
                        raw = p1s.tile([128, 512], F32, tag=f"raw{j}", bufs=2)
                        nc.scalar.activation(
                            raw[:], ps_x[j][:], IDENT, bias=b_sb[:, j : j + 1]
                        )
                        t1 = p1s.tile([128, 512], F32, tag=f"t1_{j}", bufs=2)
                        nc.vector.tensor_tensor(
                            t1[:], raw[:], cosc[:], op=mybir.AluOpType.mult
                        )
                        rot = p1s.tile([128, 512], F32, tag=f"rot{j}", bufs=2)
                        nc.vector.tensor_copy(rot[0:64, :], raw[64:128, :])
                        nc.vector.tensor_copy(rot[64:128, :], raw[0:64, :])
                        t2 = p1s.tile([128, 512], F32, tag=f"t2_{j}", bufs=2)
                        nc.vector.tensor_tensor(
                            t2[:], rot[:], sinc[:], op=mybir.AluOpType.mult
                        )
                        nc.vector.tensor_tensor(
                            dst[j][:, sl], t1[:], t2[:], op=mybir.AluOpType.add
                        )

        if DEBUG:
            for h in range(HPC):
                nc.sync.dma_start(
                    dbg_q[h * 128 : (h + 1) * 128, :], qT[h][:].bitcast(F32)
                )
                nc.sync.dma_start(
                    dbg_k[h * 128 : (h + 1) * 128, :], kT[h][:].bitcast(F32)
                )
            nc.sync.dma_start(
                dbg_v[:], v_sb[:].bitcast(F32).rearrange("p n c -> p (n c)")
            )

        # -------- Phase 2: attention per (batch, head) + pipelined gathers ----
        # -------- Phase 3: output projection (SBUF shared pool; PSUM separate)
        with ExitStack() as c2:
            p2s = c2.enter_context(tc.tile_pool(name="p2s", bufs=1))
            c2ps = ExitStack()
            p2ps = c2ps.enter_context(tc.tile_pool(name="p2ps", bufs=1, space="PSUM"))

            mask_sb = p2s.tile([128, 2048], F32, tag="mask")
            nc.sync.dma_start(mask_sb[:], maskd[:])
            # prefetch Wo during phase 2
            wo_sb = p2s.tile([128, NKT, CPC], F32R, tag="wo")
            nc.sync.dma_start(
                wo_sb[:], wo[:].rearrange("(kt p) c -> p kt c", p=128).bitcast(F32R)
            )

            for b in range(B):
                for h in range(HPC):
                    qTh = qT[h][:, b * T : (b + 1) * T]
                    kTh = kT[h][:, b * T : (b + 1) * T]
                    for half in range(2):
                        qcs = (half * 2, half * 2 + 1)
                        ps_out = p2ps.tile([128, 1024], F32, tag="pvout")
                        dsum = p2ps.tile([1, 1024], F32, tag="dsum")
                        for kt in range(TKT):
                            sc = p2ps.tile([128, 1024], F32, tag="sc", bufs=2)
                            for i, qc in enumerate(qcs):
                                nc.tensor.matmul(
                                    sc[:, i * 512 : (i + 1) * 512],
                                    lhsT=kTh[:, kt * 128 : (kt + 1) * 128],
                                    rhs=qTh[:, qc * 512 : (qc + 1) * 512],
                                    start=True,
                                    stop=True,
                                )
                            et = p2s.tile([128, 1024], F32R, tag="et", bufs=3)
                            nc.scalar.activation(et[:], sc[:], EXP, scale=SM_SCALE)
                            # denominator accumulation on PE
                            for i in range(2):
                                nc.tensor.matmul(
                                    dsum[:, i * 512 : (i + 1) * 512],
                                    lhsT=ones_sb[:],
                                    rhs=et[:, i * 512 : (i + 1) * 512],
                                    start=kt == 0,
                                    stop=kt == TKT - 1,
                                )
                            # PV accumulation (causal: key tile <= q-chunk)
                            vt = v_sb[:, b * TKT + kt, h * 128 : (h + 1) * 128]
                            ckt = kt // 4
                            for i, qc in enumerate(qcs):
                                if ckt > qc:
                                    continue
                                st = kt == 0
                                sp = kt == 4 * qc + 3
                                if ckt == qc:
                                    a = kt % 4
                                    md = p2s.tile(
                                        [128, 512], F32R, tag="md", name="md", bufs=2
                                    )
                                    nc.vector.tensor_tensor(
                                        md[:],
                                        et[:, i * 512 : (i + 1) * 512].bitcast(F32),
                                        mask_sb[:, a * 512 : (a + 1) * 512],
                                        op=mybir.AluOpType.mult,
                                    )
                                    rhs = md[:]
                                else:
                                    rhs = et[:, i * 512 : (i + 1) * 512]
                                nc.tensor.matmul(
                                    ps_out[:, i * 512 : (i + 1) * 512],
                                    lhsT=vt,
                                    rhs=rhs,
                                    start=st,
                                    stop=sp,
                                )
                        dsb = p2s.tile([1, 1024], F32, tag="dsb")
                        nc.scalar.copy(dsb[:], dsum[:])
                        rsb = p2s.tile([1, 1024], F32, tag="rsb")
                        nc.vector.reciprocal(rsb[:], dsb[:])
                        rbc = p2s.tile([128, 1024], F32, tag="rbc")
                        nc.gpsimd.partition_broadcast(rbc[:], rsb[:])
                        yt = p2s.tile([128, 1024], F32, tag="yt", bufs=2)
                        nc.vector.tensor_tensor(
                            yt[:], ps_out[:], rbc[:], op=mybir.AluOpType.mult
                        )
                        nc.sync.dma_start(
                            yg_in[b][h][:, half * 1024 : (half + 1) * 1024], yt[:]
                        )
                    # gather this instance across cores (overlaps later compute)
                    nc.gpsimd.collective_compute(
                        "AllGather",
                        mybir.AluOpType.bypass,
                        replica_groups=[list(range(NCORES))],
                        ins=[yg_in[b][h][:]],
                        outs=[yg_all[b][h][:]],
                    )
                    if DEBUG:
                        nc.sync.dma_start(
                            dbg_y[h * 128 : (h + 1) * 128, b * T : (b + 1) * T],
                            yg_in[b][h][:],
                        )

            # ---------------- Phase 3: output projection ----------------
            c2ps.close()
            with ExitStack() as c3:
                p3ps = c3.enter_context(
                    tc.tile_pool(name="p3ps", bufs=1, space="PSUM")
                )
                for b in range(B):
                    for cq in range(T // 512):
                        sl_t = slice(cq * 512, (cq + 1) * 512)
                        ps_o = [
                            p3ps.tile(
                                [128, 512], F32, tag=f"pso{j}", name=f"pso{j}", bufs=2
                            )
                            for j in range(HPC)
                        ]
                        for par in range(2):
                            ypb = p2s.tile(
                                [128, 8, 512], F32R, tag="ypb", name="ypb", bufs=2
                            )
                            nc.sync.dma_start(
                                ypb[:],
                                yg_all[b][par][:, sl_t]
                                .rearrange("(t p) c -> p t c", p=128)
                                .bitcast(F32R),
                            )
                            for tt in range(8):
                                kt = 2 * tt + par
                                for j in range(HPC):
                                    nc.tensor.matmul(
                                        ps_o[j][:],
                                        lhsT=wo_sb[:, kt, j * 128 : (j + 1) * 128],
                                        rhs=ypb[:, tt, :],
                                        start=kt == 0,
                                        stop=kt == NKT - 1,
                                    )
                        for j in range(HPC):
                            ot = p2s.tile(
                                [128, 512], F32, tag=f"ot{j}", name=f"ot{j}", bufs=2
                            )
                            nc.scalar.activation(
                                ot[:], ps_o[j][:], IDENT, bias=bo_sb[:, j : j + 1]
                            )
                            nc.sync.dma_start(
                                oT[j * 128 : (j + 1) * 128, b * T + cq * 512 : b * T + (cq + 1) * 512],
                                ot[:],
                            )

    nc.compile()
    return nc


def _build():
    global _NC_CACHE
    if _NC_CACHE is None:
        nc = bacc.Bacc("TRN2", target_bir_lowering=False)
        _NC_CACHE = _emit(nc)
    return _NC_CACHE


def _host_prep(x, Wq, bq, Wk, bk, Wv, bv, Wo, bo):
    """Build the 8 per-core input maps."""
    xT = np.ascontiguousarray(x.reshape(BT, H).T)

    # de-interleave rope pairs: new[d] = old[2d], new[64+d] = old[2d+1]
    perm = np.concatenate([np.arange(0, HS, 2), np.arange(1, HS, 2)])

    t = np.arange(T, dtype=np.float64)
    invf = 1.0 / (10000.0 ** (np.arange(0, HS, 2, dtype=np.float64) / HS))
    ang = invf[:, None] * t[None, :]  # (64, T)
    cos64 = np.cos(ang)
    sin64 = np.sin(ang)
    cosb = np.tile(np.concatenate([cos64, cos64], axis=0).astype(np.float32), (1, B))
    sinb = np.tile(np.concatenate([-sin64, sin64], axis=0).astype(np.float32), (1, B))

    maskd = np.zeros((128, 2048), dtype=np.float32)
    p = np.arange(128)[:, None]
    j = np.arange(512)[None, :]
    for a in range(4):
        maskd[:, a * 512 : (a + 1) * 512] = (p + a * 128 <= j).astype(np.float32)

    in_maps = []
    for c in range(NCORES):
        heads = [HPC * c + i for i in range(HPC)]
        wq_c = np.concatenate(
            [Wq[:, h * HS : (h + 1) * HS][:, perm] for h in heads], axis=1
        )
        wk_c = np.concatenate(
            [Wk[:, h * HS : (h + 1) * HS][:, perm] for h in heads], axis=1
        )
        wv_c = np.concatenate([Wv[:, h * HS : (h + 1) * HS] for h in heads], axis=1)
        bq_c = np.stack([bq[h * HS : (h + 1) * HS][perm] for h in heads], axis=1)
        bk_c = np.stack([bk[h * HS : (h + 1) * HS][perm] for h in heads], axis=1)
        bv_c = np.concatenate([bv[h * HS : (h + 1) * HS] for h in heads])
        cols = slice(c * CPC, (c + 1) * CPC)
        wo_c = Wo[:, cols]
        bo_c = np.stack(
            [bo[c * CPC + i * HS : c * CPC + (i + 1) * HS] for i in range(HPC)],
            axis=1,
        )
        in_maps.append(
            {
                "xT": xT,
                "wq": np.ascontiguousarray(wq_c),
                "wk": np.ascontiguousarray(wk_c),
                "wv": np.ascontiguousarray(wv_c),
                "bq2": np.ascontiguousarray(bq_c),
                "bk2": np.ascontiguousarray(bk_c),
                "bvb": np.broadcast_to(bv_c, (128, CPC)).copy(),
                "wo": np.ascontiguousarray(wo_c),
                "bo2": np.ascontiguousarray(bo_c),
                "cosb": cosb,
                "sinb": sinb,
                "maskd": maskd,
                "onesc": np.ones((128, 1), dtype=np.float32),
            }
        )
    return in_maps


def run_sharded(inputs, trace=False):
    """Run the SPMD kernel; returns (output (B,T,H), BassKernelResults)."""
    in_maps = _host_prep(
        np.asarray(inputs["x"], dtype=np.float32),
        np.asarray(inputs["Wq"], dtype=np.float32),
        np.asarray(inputs["bq"], dtype=np.float32),
        np.asarray(inputs["Wk"], dtype=np.float32),
        np.asarray(inputs["bk"], dtype=np.float32),
        np.asarray(inputs["Wv"], dtype=np.float32),
        np.asarray(inputs["bv"], dtype=np.float32),
        np.asarray(inputs["Wo"], dtype=np.float32),
        np.asarray(inputs["bo"], dtype=np.float32),
    )
    nc = _build()
    res = run_bass_kernel_spmd(nc, in_maps, core_ids=list(range(NCORES)), trace=trace)
    o = np.empty((BT, H), dtype=np.float32)
    for c in range(NCORES):
        o[:, c * CPC : (c + 1) * CPC] = res.results[c]["oT"].T
    return o.reshape(B, T, H), res


def kernel(**inputs):
    out, _ = run_sharded(inputs, trace=False)
    return out
